# revision 7
# baseline (speedup 1.0000x reference)
"""Trainium2 Bass kernel: autoregressive graph generator (GNN encoder + LSTM + GNN decoder).

Sharding: 8-way tensor parallel over the LSTM hidden/gate dim. Each core holds
1/8 of the gate rows of W_hh (fp8, SBUF-resident) and computes its slice of the
gates; h is AllGathered (fp8) every step. The encoder SAGEConv (NF=10 -> H=2048)
composed with W_ih factors through a rank-20 bottleneck, so W_ih @ W_enc_{l,r} is
precomposed on the host and the whole x-side becomes a K=20 GEMM per step.
The mean aggregation is a fixed dense matrix A built from edge_index on the host.

Pipelined split-collective: each core's 256-dim h slice is split into two
128-dim halves. Gate m-tiles are grouped even/odd so the even group (which
produces h-half A) finishes first; AllGather-A for step t fires while the odd
group is still computing, and step t+1's gate GEMM starts on the A-half k-tiles
while AllGather-B is still in flight. The h/k dimension is permuted host-side
(all cores' A-halves first, then all B-halves) so the gathered halves are
contiguous k-tile ranges.

All layouts on device are "T-layout": [feature/hidden dim (partitions), nodes (free)].
"""

import numpy as np
import ml_dtypes

import concourse.mybir as mybir
import concourse.tile as tile
from concourse import bacc, bass_utils
from concourse.bass import ts
from concourse.masks import make_identity

BF = ml_dtypes.bfloat16
F8 = ml_dtypes.float8_e4m3

N, NF, H, NG, K = 256, 10, 2048, 20, 10
NCORES = 8
HS = H // NCORES          # 256 hidden dims per core
GD = 4 * HS               # 1024 gate rows per core
MT = GD // 128            # 8 gate m-tiles per core
KT = H // 128             # 16 h k-tiles
NT = N // 128             # 2 node tiles
GEN = NG - K              # 10 generated steps
EVENS = [0, 2, 4, 6]      # gate m-tiles fed by h-half A (hh=0)
ODDS = [1, 3, 5, 7]

_PROG = [None]


def _emit_decoder_tail(nc, pools, consts, t, vw_ps, need_next):
    """Gen-step decoder tail (after the v' GEMM): x_pred = A@v + w + b,
    x_next = [static2 | x_pred]; DMAs x_next to the output. When need_next,
    also returns (m10, x10) bf16 tiles [10, N] for the next encoder."""
    f32, bf16 = mybir.dt.float32, mybir.dt.bfloat16
    cpool, wpool, apool, gpool, spool = pools
    at, qr, st2, ident, out_d = (
        consts["at"], consts["qr"], consts["st2"],
        consts["ident"], consts["out_d"],
    )
    s = t - K
    vw_sb = wpool.tile([16, N], bf16, tag="vw", name=f"vwsb{t}")
    nc.vector.tensor_scalar_add(vw_sb[:], vw_ps[:], qr[:, s:s + 1])

    # transpose v'|w' -> non-T [N, 16] per node-tile
    vwT = []
    for j in range(NT):
        tp = spool.tile([128, 16], bf16, tag="sp", name=f"vwT{t}_{j}")
        nc.tensor.transpose(tp[:], vw_sb[:, ts(j, 128)], ident[:16, :16])
        tpsb = wpool.tile([128, 16], bf16, tag=f"vwTs{j}", name=f"vwTs{t}_{j}")
        nc.vector.tensor_copy(tpsb[:], tp[:])
        vwT.append(tpsb)

    # xa = A @ v  (per output node tile), x_next = [st2 | xa + w]
    xnext, xnb = [], []
    for j in range(NT):
        xa = spool.tile([128, 8], f32, tag="sp", name=f"xa{t}_{j}")
        for kk in range(NT):
            nc.tensor.matmul(xa[:], at[kk][:, ts(j, 128)], vwT[kk][:, 0:8],
                             start=(kk == 0), stop=(kk == NT - 1))
        xn = wpool.tile([128, NF], f32, tag=f"xn{j}", name=f"xn{t}_{j}")
        nc.vector.tensor_copy(xn[:, 0:2], st2[j][:])
        nc.vector.tensor_add(xn[:, 2:NF], xa[:], vwT[j][:, 8:16])
        nc.sync.dma_start(out_d[s, ts(j, 128), :], xn[:])
        xnext.append(xn)
        if need_next:
            xb = wpool.tile([128, NF], bf16, tag=f"xnb{j}", name=f"xnb{t}_{j}")
            nc.vector.tensor_copy(xb[:], xn[:])
            xnb.append(xb)

    if not need_next:
        return None, None

    # m10 = (A @ x_next).T  [10, N];  x10 = x_next.T  [10, N]  (both bf16)
    m10 = wpool.tile([NF, N], bf16, tag="m10", name=f"m10_{t}")
    mp = spool.tile([NF, N], f32, tag="sp", name=f"mp{t}")
    for kk in range(NT):
        nc.tensor.matmul(mp[:], xnb[kk][:], at[kk][:],
                         start=(kk == 0), stop=(kk == NT - 1))
    nc.vector.tensor_copy(m10[:], mp[:])
    x10 = wpool.tile([NF, N], bf16, tag="x10", name=f"x10_{t}")
    for kk in range(NT):
        xt = spool.tile([NF, 128], bf16, tag="sp", name=f"xt{t}_{kk}")
        nc.tensor.transpose(xt[:], xnb[kk][:], ident[:])
        nc.vector.tensor_copy(x10[:, ts(kk, 128)], xt[:])
    return m10, x10


def _emit_step(nc, pools, consts, t, outb_prev, c_prev, dpool):
    """One LSTM step with pipelined split AllGather.

    outb_prev: previous step's two AllGather DRAM outputs [H/2, N] fp8
    (k-tiles in permuted order: all cores' A halves, then all B halves).
    Their SBUF-landing DMAs are emitted HERE (step start) so the sync queue's
    dependencies resolve in emission order (no head-of-line blocking).
    Returns (c_new, (outbA', outbB')) for the next step.
    """
    f32, bf16 = mybir.dt.float32, mybir.dt.bfloat16
    fp8 = mybir.dt.float8e4
    cpool, wpool, apool, gpool, spool = pools
    whh, wc, bias, r20w = consts["whh"], consts["wc"], consts["bias"], consts["r20w"]
    wdec, wc2 = consts["wdec"], consts["wc2"]
    Sig = mybir.ActivationFunctionType.Sigmoid
    Tanh = mybir.ActivationFunctionType.Tanh
    DR = mybir.MatmulPerfMode.DoubleRow
    have_h = t > 0
    gen = t >= K
    last = t == NG - 1

    def pair_lhs(w):
        return w[:].rearrange("p (s w) -> p s w", s=2)

    # land the previous step's gathered halves in SBUF
    hvA = hvB = None
    if have_h:
        hvs = []
        for half in range(2):
            hb = wpool.tile([128, 8 * N], fp8, tag=f"hb{half}",
                            name=f"hb{t}_{half}")
            nc.sync.dma_start(hb[:].rearrange("p (a n) -> p a n", a=8),
                              outb_prev[half].rearrange("(a p) n -> p a n", p=128))
            hvs.append(hb[:].rearrange("p (a n) -> p a n", a=8))
        hvA, hvB = hvs

    # decoder v' GEMM, A half (first tensor work after AG-A of t-1 lands)
    if gen:
        vw_ps = spool.tile([16, N], f32, tag="sp", name=f"vwps{t}")
        for j in range(4):
            nc.tensor.matmul(vw_ps[:], pair_lhs(wdec[j]), hvA[:, 2 * j:2 * j + 2, :],
                             start=(j == 0), stop=False, perf_mode=DR)

    gpt = None
    if not last:
        # 4 paired PSUM banks; m-tile m lives in gpt[m//2], column half m%2
        gpt = [gpool.tile([128, 2 * N], f32, tag=f"gp{p}", bufs=1,
                          name=f"gp{t}_{p}") for p in range(4)]

    def gsl(m):
        return gpt[m // 2][:, ts(m % 2, N)]

    # warm-up x-parts: no AG dependency, fill the AG window
    if not last and not gen:
        rhs = r20w[:, t * N:(t + 1) * N]
        for m in EVENS + ODDS:
            nc.tensor.matmul(gsl(m), wc[:, ts(m, 128)], rhs,
                             start=True, stop=not have_h)

    # gate GEMM A-parts (k-tiles from AG-A of t-1)
    if not last and have_h:
        for m in EVENS + ODDS:
            for j in range(4):
                nc.tensor.matmul(
                    gsl(m), pair_lhs(whh[j])[:, :, ts(m, 128)],
                    hvA[:, 2 * j:2 * j + 2, :],
                    start=(gen and j == 0), stop=False, perf_mode=DR)

    # decoder v' B half + tail (concurrent with gate B-parts)
    m10 = x10 = None
    if gen:
        for j in range(4):
            nc.tensor.matmul(vw_ps[:], pair_lhs(wdec[4 + j]), hvB[:, 2 * j:2 * j + 2, :],
                             start=False, stop=(j == 3), perf_mode=DR)
        m10, x10 = _emit_decoder_tail(nc, pools, consts, t, vw_ps, not last)
        if last:
            return None, None

    def bcol(m):
        return bias[:, (m * NG + t):(m * NG + t + 1)]

    c_new, outb_next = [], []
    for half, ms in ((0, EVENS), (1, ODDS)):
        # gate GEMM B-parts for this half's m-tiles
        if have_h:
            for m in ms:
                for j in range(4):
                    nc.tensor.matmul(
                        gsl(m), pair_lhs(whh[4 + j])[:, :, ts(m, 128)],
                        hvB[:, 2 * j:2 * j + 2, :],
                        start=False, stop=(not gen) and (j == 3), perf_mode=DR)
        if gen:
            for m in ms:
                nc.tensor.matmul(gsl(m), wc[0:10, ts(m, 128)], m10[:],
                                 start=False, stop=False)
                nc.tensor.matmul(gsl(m), wc2[0:10, ts(m, 128)], x10[:],
                                 start=False, stop=True)

        # activations + cell update for this half
        si = apool.tile([128, N], f32, tag="si", name=f"si{t}_{half}")
        nc.scalar.activation(si[:], gsl(0 + half), Sig, bias=bcol(0 + half))
        sf = apool.tile([128, N], f32, tag="sf", name=f"sf{t}_{half}")
        nc.scalar.activation(sf[:], gsl(2 + half), Sig, bias=bcol(2 + half))
        tg = apool.tile([128, N], f32, tag="tg", name=f"tg{t}_{half}")
        nc.scalar.activation(tg[:], gsl(4 + half), Tanh, bias=bcol(4 + half))
        so = apool.tile([128, N], f32, tag="so", name=f"so{t}_{half}")
        nc.scalar.activation(so[:], gsl(6 + half), Sig, bias=bcol(6 + half))

        cn = wpool.tile([128, N], f32, tag=f"c{half}", name=f"c{t}_{half}")
        if t == 0:
            nc.vector.tensor_mul(cn[:], si[:], tg[:])          # c = sig(i)*tanh(g)
        else:
            p = apool.tile([128, N], f32, tag="p", name=f"p{t}_{half}")
            nc.vector.tensor_mul(p[:], si[:], tg[:])
            tmp = apool.tile([128, N], f32, tag="tmp", name=f"tmp{t}_{half}")
            nc.vector.tensor_mul(tmp[:], sf[:], c_prev[half][:])
            nc.vector.tensor_add(cn[:], tmp[:], p[:])
        tc2 = apool.tile([128, N], f32, tag="tc", name=f"tc{t}_{half}")
        nc.scalar.activation(tc2[:], cn[:], Tanh)
        h2h = apool.tile([128, N], fp8, tag=f"h2{half}", name=f"h2_{t}_{half}")
        nc.vector.tensor_mul(h2h[:], so[:], tc2[:])
        c_new.append(cn)

        # AllGather of this half; the SBUF landing is emitted by the next step
        if t < NG - 1:
            inb = dpool.tile([128, N], fp8, tag=f"inb{half}", name=f"inb{t}_{half}")
            outb = dpool.tile([H // 2, N], fp8, tag=f"outb{half}",
                              name=f"outb{t}_{half}")
            nc.sync.dma_start(inb[:], h2h[:])
            nc.gpsimd.collective_compute(
                "AllGather",
                mybir.AluOpType.bypass,
                replica_groups=[list(range(NCORES))],
                ins=[inb.opt()],
                outs=[outb.opt()],
            )
            outb_next.append(outb)
    return c_new, outb_next


def _build_program():
    f32, bf16 = mybir.dt.float32, mybir.dt.bfloat16
    nc = bacc.Bacc("TRN2", target_bir_lowering=False, debug=False,
                   num_devices=NCORES)

    fp8 = mybir.dt.float8e4
    whhT_d = nc.dram_tensor("whhT", [H // 2, 2 * GD], fp8,
                            kind="ExternalInput").ap()
    wcT_d = nc.dram_tensor("wcT", [20, GD], bf16, kind="ExternalInput").ap()
    wc2T_d = nc.dram_tensor("wc2T", [NF, GD], bf16, kind="ExternalInput").ap()
    bias_d = nc.dram_tensor("biases", [128, MT * NG], f32, kind="ExternalInput").ap()
    at_d = nc.dram_tensor("at", [N, N], bf16, kind="ExternalInput").ap()
    wdec_d = nc.dram_tensor("wdecT", [H // 2, 32], fp8,
                            kind="ExternalInput").ap()
    qr_d = nc.dram_tensor("qr", [16, GEN], f32, kind="ExternalInput").ap()
    r20_d = nc.dram_tensor("rhs20w", [20, K * N], bf16, kind="ExternalInput").ap()
    st2_d = nc.dram_tensor("st2", [N, 2], f32, kind="ExternalInput").ap()
    out_d = nc.dram_tensor("gen", [GEN, N, NF], f32, kind="ExternalOutput").ap()

    with tile.TileContext(nc) as tc:
        with (
            tc.tile_pool(name="const", bufs=1) as cpool,
            tc.tile_pool(name="work", bufs=2) as wpool,
            tc.tile_pool(name="act", bufs=3) as apool,
            tc.tile_pool(name="gp", bufs=1, space="PSUM") as gpool,
            tc.tile_pool(name="sp", bufs=3, space="PSUM") as spool,
            tc.tile_pool(name="dram", bufs=2, space="DRAM") as dpool,
        ):
            pools = (cpool, wpool, apool, gpool, spool)

            fp8 = mybir.dt.float8e4
            # small consts first so step 0's x-GEMM isn't stuck behind whh
            wc = cpool.tile([20, GD], bf16, tag="wc", name="wc")
            nc.sync.dma_start(wc[:], wcT_d[:])
            r20w = cpool.tile([20, K * N], bf16, tag="r20w", name="r20w")
            nc.sync.dma_start(r20w[:], r20_d[:])
            bias = cpool.tile([128, MT * NG], f32, tag="bias", name="bias")
            nc.sync.dma_start(bias[:], bias_d[:])
            wc2 = cpool.tile([NF, GD], bf16, tag="wc2", name="wc2")
            nc.sync.dma_start(wc2[:], wc2T_d[:])
            at = []
            for k in range(NT):
                a = cpool.tile([128, N], bf16, tag=f"at{k}", name=f"at{k}")
                nc.sync.dma_start(a[:], at_d[ts(k, 128), :])
                at.append(a)
            wdec = []
            for k in range(KT // 2):
                w = cpool.tile([128, 32], fp8, tag=f"wdec{k}", name=f"wdec{k}")
                nc.sync.dma_start(w[:], wdec_d[ts(k, 128), :])
                wdec.append(w)
            qr = cpool.tile([16, GEN], f32, tag="qr", name="qr")
            nc.sync.dma_start(qr[:], qr_d[:])
            st2 = []
            for j in range(NT):
                s = cpool.tile([128, 2], f32, tag=f"st2{j}", name=f"st2_{j}")
                nc.sync.dma_start(s[:], st2_d[ts(j, 128), :])
                st2.append(s)
            ident = cpool.tile([128, 128], bf16, tag="ident", name="ident")
            make_identity(nc, ident[:])
            whh = []
            for k in range(KT // 2):
                w = cpool.tile([128, 2 * GD], fp8, tag=f"whh{k}", name=f"whh{k}")
                nc.sync.dma_start(w[:], whhT_d[ts(k, 128), :])
                whh.append(w)

            consts = dict(whh=whh, wc=wc, wc2=wc2, bias=bias, at=at, wdec=wdec,
                          qr=qr, r20w=r20w, st2=st2, ident=ident, out_d=out_d)

            outb_prev = None
            c_prev = None
            for t in range(NG):
                c_prev, outb_prev = _emit_step(nc, pools, consts, t, outb_prev,
                                               c_prev, dpool)
    nc.compile()
    return nc


def _host_tensors(inputs):
    """All host-side preprocessing: A matrix, weight composition, per-core shards."""
    f32 = np.float32
    kg = np.asarray(inputs["known_graphs"], f32)
    ei = np.asarray(inputs["edge_index"])
    W_enc_l = np.asarray(inputs["W_enc_l"], f32)
    b_enc_l = np.asarray(inputs["b_enc_l"], f32)
    W_enc_r = np.asarray(inputs["W_enc_r"], f32)
    pos = np.asarray(inputs["pos_emb"], f32)
    W_ih = np.asarray(inputs["W_ih"], f32)
    W_hh = np.asarray(inputs["W_hh"], f32)
    b_ih = np.asarray(inputs["b_ih"], f32)
    b_hh = np.asarray(inputs["b_hh"], f32)
    W_dec_l = np.asarray(inputs["W_dec_l"], f32)
    b_dec_l = np.asarray(inputs["b_dec_l"], f32)
    W_dec_r = np.asarray(inputs["W_dec_r"], f32)

    src, dst = np.asarray(ei[0]), np.asarray(ei[1])
    C = np.zeros((N, N), np.float64)
    np.add.at(C, (dst, src), 1.0)
    cnt = C.sum(1)
    A = (C / np.maximum(cnt, 1.0)[:, None]).astype(f32)

    c64 = np.float64
    Wc1 = W_ih.astype(c64) @ W_enc_l.astype(c64)          # [4H, NF]
    Wc2 = W_ih.astype(c64) @ W_enc_r.astype(c64)
    Wc = np.concatenate([Wc1, Wc2], 1)                    # [4H, 20]
    # bias_t = W_ih @ (b_enc_l + pe_t) + b_ih + b_hh  -> [NG, 4H]
    bias_all = (W_ih.astype(c64) @ (b_enc_l.astype(c64)[:, None] + pos.astype(c64).T)).T \
        + b_ih.astype(c64) + b_hh.astype(c64)
    bias_all = bias_all.astype(f32)
    # decoder pe folds: [16, GEN]
    qr = np.concatenate([
        (pos[K:NG].astype(c64) @ W_dec_l.T.astype(c64)).T,
        (pos[K:NG].astype(c64) @ W_dec_r.T.astype(c64)).T
        + b_dec_l.astype(c64)[:, None],
    ], 0).astype(f32)

    # warm-up rhs20: [20, K*N], col index t*N + i
    mean_w = np.einsum("ij,tjf->tif", A.astype(c64), kg.astype(c64))  # [K, N, NF]
    r20w = np.concatenate([
        np.transpose(mean_w, (2, 0, 1)).reshape(NF, -1),
        np.transpose(kg.astype(c64), (2, 0, 1)).reshape(NF, -1),
    ], 0).astype(f32)

    # k-dim permutation matching the split-gather layout: all cores' first
    # 128-dim halves, then all second halves
    PI = np.concatenate([np.arange(c * HS, c * HS + 128) for c in range(NCORES)]
                        + [np.arange(c * HS + 128, (c + 1) * HS)
                           for c in range(NCORES)])

    # DoubleRow pair packing: [KT/2 * 128, 2*cols], row j*128+p holds
    # k-tiles (2j, 2j+1) side by side along the free dim
    def pack_pairs(wT):  # wT [H, cols] -> [H/2, 2*cols]
        cols = wT.shape[1]
        return np.ascontiguousarray(
            wT.reshape(KT // 2, 2, 128, cols).transpose(0, 2, 1, 3)
            .reshape(H // 2, 2 * cols))

    wdecT = np.concatenate([W_dec_l, W_dec_r], 0).T[PI]    # [H, 16] permuted
    shared = {
        "at": np.ascontiguousarray(A.T).astype(BF),
        "wdecT": pack_pairs(wdecT).astype(F8),
        "qr": np.ascontiguousarray(qr),
        "rhs20w": np.ascontiguousarray(r20w).astype(BF),
        "st2": np.ascontiguousarray(kg[-1, :, :2]),
    }

    in_maps = []
    for c in range(NCORES):
        idx = np.concatenate([np.arange(g * H + c * HS, g * H + (c + 1) * HS)
                              for g in range(4)])
        whhT = pack_pairs(W_hh[idx, :].T[PI]).astype(F8)              # [H/2, 2GD]
        wcT = np.ascontiguousarray(Wc[idx, :].T).astype(BF)           # [20, GD]
        wc2T = np.ascontiguousarray(Wc[idx, NF:].T).astype(BF)        # [NF, GD]
        bc = bias_all[:, idx].T                                       # [GD, NG]
        bt = np.ascontiguousarray(
            bc.reshape(MT, 128, NG).transpose(1, 0, 2).reshape(128, MT * NG))
        in_maps.append({
            "whhT": whhT, "wcT": wcT, "wc2T": wc2T, "biases": bt, **shared,
        })
    return in_maps


def kernel(**inputs):
    if _PROG[0] is None:
        _PROG[0] = _build_program()
    nc = _PROG[0]
    in_maps = _host_tensors(inputs)
    res = bass_utils.run_bass_kernel_spmd(
        nc, in_maps, core_ids=list(range(NCORES)))
    return np.ascontiguousarray(res.results[0]["gen"]).astype(np.float32)


# exposed for test.py profiling
def run_profiled(inputs, **kwargs):
    if _PROG[0] is None:
        _PROG[0] = _build_program()
    in_maps = _host_tensors(inputs)
    return bass_utils.run_bass_kernel_spmd(
        _PROG[0], in_maps, core_ids=list(range(NCORES)), **kwargs)


# revision 9
# speedup vs baseline: 1.1198x; 1.1198x over previous
"""Trainium2 Bass kernel: autoregressive graph generator (GNN encoder + LSTM + GNN decoder).

Sharding: 8-way tensor parallel over the LSTM hidden/gate dim. Each core holds
1/8 of the gate rows of W_hh (fp8, SBUF-resident) and computes its slice of the
gates; h is AllGathered (fp8) every step. The encoder SAGEConv (NF=10 -> H=2048)
composed with W_ih factors through a rank-20 bottleneck, so W_ih @ W_enc_{l,r} is
precomposed on the host and the whole x-side becomes a K=20 GEMM per step.
The mean aggregation is a fixed dense matrix A built from edge_index on the host.

Pipelined split-collective: each core's 256-dim h slice is split into two
128-dim halves. Gate m-tiles are grouped even/odd so the even group (which
produces h-half A) finishes first; AllGather-A for step t fires while the odd
group is still computing, and step t+1's gate GEMM starts on the A-half k-tiles
while AllGather-B is still in flight. The h/k dimension is permuted host-side
(all cores' A-halves first, then all B-halves) so the gathered halves are
contiguous k-tile ranges.

All layouts on device are "T-layout": [feature/hidden dim (partitions), nodes (free)].
"""

import numpy as np
import ml_dtypes

import concourse.mybir as mybir
import concourse.tile as tile
from concourse import bacc, bass_utils
from concourse.bass import ts
from concourse.masks import make_identity

BF = ml_dtypes.bfloat16
F8 = ml_dtypes.float8_e4m3

N, NF, H, NG, K = 256, 10, 2048, 20, 10
NCORES = 8
HS = H // NCORES          # 256 hidden dims per core
GD = 4 * HS               # 1024 gate rows per core
MT = GD // 128            # 8 gate m-tiles per core
KT = H // 128             # 16 h k-tiles
NT = N // 128             # 2 node tiles
GEN = NG - K              # 10 generated steps
EVENS = [0, 2, 4, 6]      # gate m-tiles fed by h-half A (hh=0)
ODDS = [1, 3, 5, 7]

_PROG = [None]


def _emit_decoder_tail(nc, pools, consts, t, vw_ps, need_next):
    """Gen-step decoder tail (after the v' GEMM): x_pred = A@v + w + b,
    x_next = [static2 | x_pred]; DMAs x_next to the output. When need_next,
    also returns (m10, x10) bf16 tiles [10, N] for the next encoder."""
    f32, bf16 = mybir.dt.float32, mybir.dt.bfloat16
    cpool, wpool, apool, gpool, spool = pools
    at, qr, st2, ident, out_d = (
        consts["at"], consts["qr"], consts["st2"],
        consts["ident"], consts["out_d"],
    )
    s = t - K
    vw_sb = wpool.tile([16, N], bf16, tag="vw", name=f"vwsb{t}")
    nc.vector.tensor_scalar_add(vw_sb[:], vw_ps[:], qr[:, s:s + 1])

    # transpose v'|w' -> non-T [N, 16] per node-tile
    vwT = []
    for j in range(NT):
        tp = spool.tile([128, 16], bf16, tag="sp", name=f"vwT{t}_{j}")
        nc.tensor.transpose(tp[:], vw_sb[:, ts(j, 128)], ident[:16, :16])
        tpsb = wpool.tile([128, 16], bf16, tag=f"vwTs{j}", name=f"vwTs{t}_{j}")
        nc.vector.tensor_copy(tpsb[:], tp[:])
        vwT.append(tpsb)

    # xa = A @ v  (per output node tile), x_next = [st2 | xa + w]
    xnext, xnb = [], []
    for j in range(NT):
        xa = spool.tile([128, 8], f32, tag="sp", name=f"xa{t}_{j}")
        for kk in range(NT):
            nc.tensor.matmul(xa[:], at[kk][:, ts(j, 128)], vwT[kk][:, 0:8],
                             start=(kk == 0), stop=(kk == NT - 1))
        xn = wpool.tile([128, NF], f32, tag=f"xn{j}", name=f"xn{t}_{j}")
        nc.vector.tensor_copy(xn[:, 0:2], st2[j][:])
        nc.vector.tensor_add(xn[:, 2:NF], xa[:], vwT[j][:, 8:16])
        nc.sync.dma_start(out_d[s, ts(j, 128), :], xn[:])
        xnext.append(xn)
        if need_next:
            xb = wpool.tile([128, NF], bf16, tag=f"xnb{j}", name=f"xnb{t}_{j}")
            nc.vector.tensor_copy(xb[:], xn[:])
            xnb.append(xb)

    if not need_next:
        return None, None

    # m10 = (A @ x_next).T  [10, N];  x10 = x_next.T  [10, N]  (both bf16)
    m10 = wpool.tile([NF, N], bf16, tag="m10", name=f"m10_{t}")
    mp = spool.tile([NF, N], f32, tag="sp", name=f"mp{t}")
    for kk in range(NT):
        nc.tensor.matmul(mp[:], xnb[kk][:], at[kk][:],
                         start=(kk == 0), stop=(kk == NT - 1))
    nc.vector.tensor_copy(m10[:], mp[:])
    x10 = wpool.tile([NF, N], bf16, tag="x10", name=f"x10_{t}")
    for kk in range(NT):
        xt = spool.tile([NF, 128], bf16, tag="sp", name=f"xt{t}_{kk}")
        nc.tensor.transpose(xt[:], xnb[kk][:], ident[:])
        nc.vector.tensor_copy(x10[:, ts(kk, 128)], xt[:])
    return m10, x10


def _emit_step(nc, pools, consts, t, outb_prev, c_prev, dpool):
    """One LSTM step with pipelined split AllGather.

    outb_prev: previous step's two AllGather DRAM outputs [H/2, N] fp8
    (k-tiles in permuted order: all cores' A halves, then all B halves).
    Their SBUF-landing DMAs are emitted HERE (step start) so the sync queue's
    dependencies resolve in emission order (no head-of-line blocking).
    Returns (c_new, (outbA', outbB')) for the next step.
    """
    f32, bf16 = mybir.dt.float32, mybir.dt.bfloat16
    fp8 = mybir.dt.float8e4
    cpool, wpool, apool, gpool, spool = pools
    whh, wc, bias, r20w = consts["whh"], consts["wc"], consts["bias"], consts["r20w"]
    wdec, wc2 = consts["wdec"], consts["wc2"]
    Sig = mybir.ActivationFunctionType.Sigmoid
    Tanh = mybir.ActivationFunctionType.Tanh
    DR = mybir.MatmulPerfMode.DoubleRow
    have_h = t > 0
    gen = t >= K
    last = t == NG - 1

    def pair_lhs(w):
        return w[:].rearrange("p (s w) -> p s w", s=2)

    # land the previous step's gathered halves in SBUF (chunked so the GEMM
    # can start after the first chunk)
    hvA = hvB = None
    if have_h:
        hvs = []
        for half in range(2):
            hb = wpool.tile([128, 8 * N], fp8, tag=f"hb{half}",
                            name=f"hb{t}_{half}")
            h3 = hb[:].rearrange("p (a n) -> p a n", a=8)
            o3 = outb_prev[half].rearrange("(a p) n -> p a n", p=128)
            for c in range(2):
                nc.sync.dma_start(h3[:, 4 * c:4 * c + 4, :],
                                  o3[:, 4 * c:4 * c + 4, :])
            hvs.append(h3)
        hvA, hvB = hvs

    gpt = None
    if not last:
        # 4 paired PSUM banks; m-tile m lives in gpt[m//2], column half m%2.
        # All GEMM loops run j-outer / m-inner so consecutive matmuls cycle
        # across banks (avoids the PSUM same-region read-modify-write stall).
        gpt = [gpool.tile([128, 2 * N], f32, tag=f"gp{p}", bufs=1,
                          name=f"gp{t}_{p}") for p in range(4)]

    def gsl(m):
        return gpt[m // 2][:, ts(m % 2, N)]

    # warm-up x-parts: no AG dependency, fill the AG window
    if not last and not gen:
        rhs = r20w[:, t * N:(t + 1) * N]
        for m in EVENS + ODDS:
            nc.tensor.matmul(gsl(m), wc[:, ts(m, 128)], rhs,
                             start=True, stop=not have_h)

    # gate GEMM A-parts (k-tiles from AG-A of t-1)
    if not last and have_h:
        for j in range(4):
            for m in EVENS + ODDS:
                nc.tensor.matmul(
                    gsl(m), pair_lhs(whh[j])[:, :, ts(m, 128)],
                    hvA[:, 2 * j:2 * j + 2, :],
                    start=(gen and j == 0), stop=False, perf_mode=DR)

    # decoder v' GEMM A half — after the gate A-parts (off critical path)
    m10 = x10 = None
    if gen:
        vw_ps = spool.tile([16, N], f32, tag="sp", name=f"vwps{t}")
        for j in range(4):
            nc.tensor.matmul(vw_ps[:], pair_lhs(wdec[j]), hvA[:, 2 * j:2 * j + 2, :],
                             start=(j == 0), stop=False, perf_mode=DR)

    def bcol(m):
        return bias[:, (m * NG + t):(m * NG + t + 1)]

    c_new, outb_next = [], []
    for half, ms in ((0, EVENS), (1, ODDS)):
        # gate GEMM B-parts for this half's m-tiles
        if have_h and not last:
            for j in range(4):
                for m in ms:
                    nc.tensor.matmul(
                        gsl(m), pair_lhs(whh[4 + j])[:, :, ts(m, 128)],
                        hvB[:, 2 * j:2 * j + 2, :],
                        start=False, stop=(not gen) and (j == 3), perf_mode=DR)
        if gen and half == 0:
            # decoder v' B half + tail, before the evens' x-parts need m10/x10
            for j in range(4):
                nc.tensor.matmul(vw_ps[:], pair_lhs(wdec[4 + j]),
                                 hvB[:, 2 * j:2 * j + 2, :],
                                 start=False, stop=(j == 3), perf_mode=DR)
            m10, x10 = _emit_decoder_tail(nc, pools, consts, t, vw_ps, not last)
            if last:
                return None, None
        if gen:
            for m in ms:
                nc.tensor.matmul(gsl(m), wc[0:10, ts(m, 128)], m10[:],
                                 start=False, stop=False)
            for m in ms:
                nc.tensor.matmul(gsl(m), wc2[0:10, ts(m, 128)], x10[:],
                                 start=False, stop=True)

        # activations + cell update for this half
        si = apool.tile([128, N], f32, tag="si", name=f"si{t}_{half}")
        nc.scalar.activation(si[:], gsl(0 + half), Sig, bias=bcol(0 + half))
        sf = apool.tile([128, N], f32, tag="sf", name=f"sf{t}_{half}")
        nc.scalar.activation(sf[:], gsl(2 + half), Sig, bias=bcol(2 + half))
        tg = apool.tile([128, N], f32, tag="tg", name=f"tg{t}_{half}")
        nc.scalar.activation(tg[:], gsl(4 + half), Tanh, bias=bcol(4 + half))
        so = apool.tile([128, N], f32, tag="so", name=f"so{t}_{half}")
        nc.scalar.activation(so[:], gsl(6 + half), Sig, bias=bcol(6 + half))

        cn = wpool.tile([128, N], f32, tag=f"c{half}", name=f"c{t}_{half}")
        if t == 0:
            nc.vector.tensor_mul(cn[:], si[:], tg[:])          # c = sig(i)*tanh(g)
        else:
            p = apool.tile([128, N], f32, tag="p", name=f"p{t}_{half}")
            nc.vector.tensor_mul(p[:], si[:], tg[:])
            tmp = apool.tile([128, N], f32, tag="tmp", name=f"tmp{t}_{half}")
            nc.vector.tensor_mul(tmp[:], sf[:], c_prev[half][:])
            nc.vector.tensor_add(cn[:], tmp[:], p[:])
        tc2 = apool.tile([128, N], f32, tag="tc", name=f"tc{t}_{half}")
        nc.scalar.activation(tc2[:], cn[:], Tanh)
        h2h = apool.tile([128, N], fp8, tag=f"h2{half}", name=f"h2_{t}_{half}")
        nc.vector.tensor_mul(h2h[:], so[:], tc2[:])
        c_new.append(cn)

        # AllGather of this half; the SBUF landing is emitted by the next step
        if t < NG - 1:
            inb = dpool.tile([128, N], fp8, tag=f"inb{half}", name=f"inb{t}_{half}")
            outb = dpool.tile([H // 2, N], fp8, tag=f"outb{half}",
                              name=f"outb{t}_{half}")
            nc.sync.dma_start(inb[:], h2h[:])
            nc.gpsimd.collective_compute(
                "AllGather",
                mybir.AluOpType.bypass,
                replica_groups=[list(range(NCORES))],
                ins=[inb.opt()],
                outs=[outb.opt()],
            )
            outb_next.append(outb)
    return c_new, outb_next


def _build_program():
    f32, bf16 = mybir.dt.float32, mybir.dt.bfloat16
    nc = bacc.Bacc("TRN2", target_bir_lowering=False, debug=False,
                   num_devices=NCORES)

    fp8 = mybir.dt.float8e4
    whhT_d = nc.dram_tensor("whhT", [H // 2, 2 * GD], fp8,
                            kind="ExternalInput").ap()
    wcT_d = nc.dram_tensor("wcT", [20, GD], bf16, kind="ExternalInput").ap()
    wc2T_d = nc.dram_tensor("wc2T", [NF, GD], bf16, kind="ExternalInput").ap()
    bias_d = nc.dram_tensor("biases", [128, MT * NG], f32, kind="ExternalInput").ap()
    at_d = nc.dram_tensor("at", [N, N], bf16, kind="ExternalInput").ap()
    wdec_d = nc.dram_tensor("wdecT", [H // 2, 32], fp8,
                            kind="ExternalInput").ap()
    qr_d = nc.dram_tensor("qr", [16, GEN], f32, kind="ExternalInput").ap()
    r20_d = nc.dram_tensor("rhs20w", [20, K * N], bf16, kind="ExternalInput").ap()
    st2_d = nc.dram_tensor("st2", [N, 2], f32, kind="ExternalInput").ap()
    out_d = nc.dram_tensor("gen", [GEN, N, NF], f32, kind="ExternalOutput").ap()

    with tile.TileContext(nc) as tc:
        with (
            tc.tile_pool(name="const", bufs=1) as cpool,
            tc.tile_pool(name="work", bufs=2) as wpool,
            tc.tile_pool(name="act", bufs=3) as apool,
            tc.tile_pool(name="gp", bufs=1, space="PSUM") as gpool,
            tc.tile_pool(name="sp", bufs=3, space="PSUM") as spool,
            tc.tile_pool(name="dram", bufs=2, space="DRAM") as dpool,
        ):
            pools = (cpool, wpool, apool, gpool, spool)

            fp8 = mybir.dt.float8e4
            # small consts first so step 0's x-GEMM isn't stuck behind whh
            wc = cpool.tile([20, GD], bf16, tag="wc", name="wc")
            nc.sync.dma_start(wc[:], wcT_d[:])
            r20w = cpool.tile([20, K * N], bf16, tag="r20w", name="r20w")
            nc.sync.dma_start(r20w[:], r20_d[:])
            bias = cpool.tile([128, MT * NG], f32, tag="bias", name="bias")
            nc.sync.dma_start(bias[:], bias_d[:])
            wc2 = cpool.tile([NF, GD], bf16, tag="wc2", name="wc2")
            nc.sync.dma_start(wc2[:], wc2T_d[:])
            at = []
            for k in range(NT):
                a = cpool.tile([128, N], bf16, tag=f"at{k}", name=f"at{k}")
                nc.sync.dma_start(a[:], at_d[ts(k, 128), :])
                at.append(a)
            wdec = []
            for k in range(KT // 2):
                w = cpool.tile([128, 32], fp8, tag=f"wdec{k}", name=f"wdec{k}")
                nc.sync.dma_start(w[:], wdec_d[ts(k, 128), :])
                wdec.append(w)
            qr = cpool.tile([16, GEN], f32, tag="qr", name="qr")
            nc.sync.dma_start(qr[:], qr_d[:])
            st2 = []
            for j in range(NT):
                s = cpool.tile([128, 2], f32, tag=f"st2{j}", name=f"st2_{j}")
                nc.sync.dma_start(s[:], st2_d[ts(j, 128), :])
                st2.append(s)
            ident = cpool.tile([128, 128], bf16, tag="ident", name="ident")
            make_identity(nc, ident[:])
            whh = []
            for k in range(KT // 2):
                w = cpool.tile([128, 2 * GD], fp8, tag=f"whh{k}", name=f"whh{k}")
                nc.sync.dma_start(w[:], whhT_d[ts(k, 128), :])
                whh.append(w)

            consts = dict(whh=whh, wc=wc, wc2=wc2, bias=bias, at=at, wdec=wdec,
                          qr=qr, r20w=r20w, st2=st2, ident=ident, out_d=out_d)

            outb_prev = None
            c_prev = None
            for t in range(NG):
                c_prev, outb_prev = _emit_step(nc, pools, consts, t, outb_prev,
                                               c_prev, dpool)
    nc.compile()
    return nc


def _host_tensors(inputs):
    """All host-side preprocessing: A matrix, weight composition, per-core shards."""
    f32 = np.float32
    kg = np.asarray(inputs["known_graphs"], f32)
    ei = np.asarray(inputs["edge_index"])
    W_enc_l = np.asarray(inputs["W_enc_l"], f32)
    b_enc_l = np.asarray(inputs["b_enc_l"], f32)
    W_enc_r = np.asarray(inputs["W_enc_r"], f32)
    pos = np.asarray(inputs["pos_emb"], f32)
    W_ih = np.asarray(inputs["W_ih"], f32)
    W_hh = np.asarray(inputs["W_hh"], f32)
    b_ih = np.asarray(inputs["b_ih"], f32)
    b_hh = np.asarray(inputs["b_hh"], f32)
    W_dec_l = np.asarray(inputs["W_dec_l"], f32)
    b_dec_l = np.asarray(inputs["b_dec_l"], f32)
    W_dec_r = np.asarray(inputs["W_dec_r"], f32)

    src, dst = np.asarray(ei[0]), np.asarray(ei[1])
    C = np.zeros((N, N), np.float64)
    np.add.at(C, (dst, src), 1.0)
    cnt = C.sum(1)
    A = (C / np.maximum(cnt, 1.0)[:, None]).astype(f32)

    c64 = np.float64
    Wc1 = W_ih.astype(c64) @ W_enc_l.astype(c64)          # [4H, NF]
    Wc2 = W_ih.astype(c64) @ W_enc_r.astype(c64)
    Wc = np.concatenate([Wc1, Wc2], 1)                    # [4H, 20]
    # bias_t = W_ih @ (b_enc_l + pe_t) + b_ih + b_hh  -> [NG, 4H]
    bias_all = (W_ih.astype(c64) @ (b_enc_l.astype(c64)[:, None] + pos.astype(c64).T)).T \
        + b_ih.astype(c64) + b_hh.astype(c64)
    bias_all = bias_all.astype(f32)
    # decoder pe folds: [16, GEN]
    qr = np.concatenate([
        (pos[K:NG].astype(c64) @ W_dec_l.T.astype(c64)).T,
        (pos[K:NG].astype(c64) @ W_dec_r.T.astype(c64)).T
        + b_dec_l.astype(c64)[:, None],
    ], 0).astype(f32)

    # warm-up rhs20: [20, K*N], col index t*N + i
    mean_w = np.einsum("ij,tjf->tif", A.astype(c64), kg.astype(c64))  # [K, N, NF]
    r20w = np.concatenate([
        np.transpose(mean_w, (2, 0, 1)).reshape(NF, -1),
        np.transpose(kg.astype(c64), (2, 0, 1)).reshape(NF, -1),
    ], 0).astype(f32)

    # k-dim permutation matching the split-gather layout: all cores' first
    # 128-dim halves, then all second halves
    PI = np.concatenate([np.arange(c * HS, c * HS + 128) for c in range(NCORES)]
                        + [np.arange(c * HS + 128, (c + 1) * HS)
                           for c in range(NCORES)])

    # DoubleRow pair packing: [KT/2 * 128, 2*cols], row j*128+p holds
    # k-tiles (2j, 2j+1) side by side along the free dim
    def pack_pairs(wT):  # wT [H, cols] -> [H/2, 2*cols]
        cols = wT.shape[1]
        return np.ascontiguousarray(
            wT.reshape(KT // 2, 2, 128, cols).transpose(0, 2, 1, 3)
            .reshape(H // 2, 2 * cols))

    wdecT = np.concatenate([W_dec_l, W_dec_r], 0).T[PI]    # [H, 16] permuted
    shared = {
        "at": np.ascontiguousarray(A.T).astype(BF),
        "wdecT": pack_pairs(wdecT).astype(F8),
        "qr": np.ascontiguousarray(qr),
        "rhs20w": np.ascontiguousarray(r20w).astype(BF),
        "st2": np.ascontiguousarray(kg[-1, :, :2]),
    }

    in_maps = []
    for c in range(NCORES):
        idx = np.concatenate([np.arange(g * H + c * HS, g * H + (c + 1) * HS)
                              for g in range(4)])
        whhT = pack_pairs(W_hh[idx, :].T[PI]).astype(F8)              # [H/2, 2GD]
        wcT = np.ascontiguousarray(Wc[idx, :].T).astype(BF)           # [20, GD]
        wc2T = np.ascontiguousarray(Wc[idx, NF:].T).astype(BF)        # [NF, GD]
        bc = bias_all[:, idx].T                                       # [GD, NG]
        bt = np.ascontiguousarray(
            bc.reshape(MT, 128, NG).transpose(1, 0, 2).reshape(128, MT * NG))
        in_maps.append({
            "whhT": whhT, "wcT": wcT, "wc2T": wc2T, "biases": bt, **shared,
        })
    return in_maps


def kernel(**inputs):
    if _PROG[0] is None:
        _PROG[0] = _build_program()
    nc = _PROG[0]
    in_maps = _host_tensors(inputs)
    res = bass_utils.run_bass_kernel_spmd(
        nc, in_maps, core_ids=list(range(NCORES)))
    return np.ascontiguousarray(res.results[0]["gen"]).astype(np.float32)


# exposed for test.py profiling
def run_profiled(inputs, **kwargs):
    if _PROG[0] is None:
        _PROG[0] = _build_program()
    in_maps = _host_tensors(inputs)
    return bass_utils.run_bass_kernel_spmd(
        _PROG[0], in_maps, core_ids=list(range(NCORES)), **kwargs)
